# revision 1
# baseline (speedup 1.0000x reference)
"""DualMemorySystem Trainium2 kernel — 8-core SPMD (batch x 4 row-bands).

Per core: one (b, 32-row out band). Convolution form of unfold/attention/fold:
  sim = conv(x, mem)      -> p matmuls per 4-row window, K=(kernel-row, c), fp16
  att = softmax_m(sim)    -> exp (ACT), ones-matmul partition sum,
                             reciprocal_approx (DVE) + multiplies (DVE+GpSimd)
  R_i = conv_x(att, mem)  -> matmuls over col-shifted att replicas, fp16
  out = fold_y(R)         -> log-tree of shifted adds (in-place), partition
                             moves via SBUF->SBUF DMA
  fusion: pooled partials via STT accum -> host MLP between launches ->
          tiny phase-B kernel applies softmax weights + combines.

Software-pipelined emission: branch order (p=7, p=5, p=3); per branch the
PE stream is conv1(k) [denominator matmuls trail at a 2-window lag], then
conv2(k-1), so the PE never waits on the current branch's softmax chain.
Double-buffered PSUM pools (8 banks exactly). DMAs ride only the sync and
gpsimd queues mid-kernel; large weight loads are deferred to just before
first use so the x8 image loads own the DMA engines at startup.

Hardware constraints baked in (probed): matmul dst partition base must be 0;
engines cannot remap partitions (only DMA/PE move data across partitions);
no divide ALU on DVE; fp32r matmul needs N>=256; DVE ops need 32-aligned
partition bases; only gpsimd DMAs may cast dtypes; DMA issue costs ~0.7us
of issuing-engine time regardless of size (so batch DMAs).
"""
import numpy as np
from contextlib import ExitStack

import concourse.bass as bass
import concourse.bacc as bacc
import concourse.tile as tile
from concourse import mybir
from concourse.bass_utils import run_bass_kernel_spmd

F32 = mybir.dt.float32
F32R = mybir.dt.float32r
F16 = mybir.dt.float16

B, C, H, W = 2, 16, 128, 128
PS = (3, 5, 7)
PADS = (1, 2, 3)
NBG, NTG = 64, 8
NCORES = 8
NBANDS = 4
BH = H // NBANDS            # 32 out rows per core
RX = 38                     # x replica rows per core
CX = 134                    # x cols with halo (128 + 6)
RA = 38                     # max att rows (32 + 2*padmax)
RAL = 40                    # R sbuf rows (fold-tree halo)
RAS = [BH + 2 * p for p in PADS]   # att rows per branch: 34, 36, 38
SEQ = (2, 1, 0)             # branch processing order: p=7, 5, 3
W2BASE = [0, 2, 5]

_CACHE = {}


def _windows(ra):
    return [(r0, min(4, ra - r0)) for r0 in range(0, ra, 4)]


def _build_A():
    nc = bacc.Bacc("TRN2", target_bir_lowering=False, debug=False,
                   num_devices=NCORES)

    d_x8bg = nc.dram_tensor("x8bg", [112, RX, CX], F16, kind="ExternalInput")
    d_x8tg = nc.dram_tensor("x8tg", [112, RX, CX], F16, kind="ExternalInput")
    d_hug = nc.dram_tensor("hug", [3, RA, W], F32, kind="ExternalInput")
    d_rdiv = nc.dram_tensor("rdiv", [96, BH, W], F16, kind="ExternalInput")
    d_ones = nc.dram_tensor("oneslhs", [73, 72], F32, kind="ExternalInput")
    d_w1all = nc.dram_tensor("w1all", [112, 1080], F16, kind="ExternalInput")
    d_selw = nc.dram_tensor("selw", [128, 3, 16], F16, kind="ExternalInput")
    d_w2bg = nc.dram_tensor("w2bg", [128, 9, 128], F16, kind="ExternalInput")
    d_w2tg = nc.dram_tensor("w2tg", [64, 3, 128], F16, kind="ExternalInput")
    d_fdiv = nc.dram_tensor("fdiv_out", [96, BH, W], F16,
                            kind="ExternalOutput")
    d_pool = nc.dram_tensor("pool_out", [32], F32, kind="ExternalOutput")

    with tile.TileContext(nc) as tc, ExitStack() as ctx:
        P = ctx.enter_context(tc.tile_pool(name="persist", bufs=1))
        pE = ctx.enter_context(tc.tile_pool(name="epool", bufs=2))
        pEtg = ctx.enter_context(tc.tile_pool(name="etg", bufs=2))
        pRcp = ctx.enter_context(tc.tile_pool(name="rcp", bufs=2))
        pS = ctx.enter_context(tc.tile_pool(name="spool", bufs=2))
        pR = ctx.enter_context(tc.tile_pool(name="rsb", bufs=4))
        pQ = ctx.enter_context(tc.tile_pool(name="qpool", bufs=2))
        pQo = ctx.enter_context(tc.tile_pool(name="qout", bufs=1))
        ps_c1bg = ctx.enter_context(
            tc.tile_pool(name="pc1bg", bufs=2, space=bass.MemorySpace.PSUM))
        ps_c1tg = ctx.enter_context(
            tc.tile_pool(name="pc1tg", bufs=1, space=bass.MemorySpace.PSUM))
        ps_den = ctx.enter_context(
            tc.tile_pool(name="pden", bufs=2, space=bass.MemorySpace.PSUM))
        ps_c2 = ctx.enter_context(
            tc.tile_pool(name="pc2", bufs=3, space=bass.MemorySpace.PSUM))

        # ---- startup-critical loads only: x8 images + first-branch weights
        x8 = {}
        w1all = P.tile([112, 1080], F16, tag="w1all")
        nc.scalar.dma_start(w1all[:, 512:960], d_w1all[:, 512:960])
        nc.scalar.dma_start(w1all[:, 1024:1080], d_w1all[:, 1024:1080])
        t = P.tile([112, RX, CX], F16, tag="x8bg")
        nc.sync.dma_start(t[:, 0:20, :], d_x8bg[:, 0:20, :])
        x8["bg"] = t
        t = P.tile([112, RX, CX], F16, tag="x8tg")
        nc.gpsimd.dma_start(t[:, 0:20, :], d_x8tg[:, 0:20, :])
        x8["tg"] = t
        nc.sync.dma_start(x8["bg"][:, 20:RX, :], d_x8bg[:, 20:RX, :])
        nc.gpsimd.dma_start(x8["tg"][:, 20:RX, :], d_x8tg[:, 20:RX, :])
        nc.scalar.dma_start(w1all[:, 0:512], d_w1all[:, 0:512])
        nc.scalar.dma_start(w1all[:, 960:1024], d_w1all[:, 960:1024])
        # packed col offsets: bg0,bg1,bg2 then tg0,tg1,tg2
        W1OFF = {("bg", 0): 0, ("bg", 1): 192, ("bg", 2): 512,
                 ("tg", 0): 960, ("tg", 1): 984, ("tg", 2): 1024}

        def w1ap(s, n, j):
            M = NBG if s == "bg" else NTG
            off = W1OFF[(s, n)] + j * M
            return w1all[0:16 * PS[n], off:off + M]

        ones_l = P.tile([73, 72], F32R, tag="ones")
        nc.gpsimd.dma_start(ones_l[:], d_ones[:])

        fdiv = P.tile([96, BH, W], F16, tag="fdiv")
        pacc = P.tile([96, 1], F32, tag="pacc")
        late = {}   # deferred persistent tiles: w2bg, w2tg, rdiv

        state = {}

        def load_late():
            t = P.tile([128, 9, 128], F16, tag="w2bg")
            nc.scalar.dma_start(t[:], d_w2bg[:])
            late["w2bg"] = t
            t = P.tile([64, 3, 128], F16, tag="w2tg")
            nc.scalar.dma_start(t[:], d_w2tg[:])
            late["w2tg"] = t
            t = P.tile([96, BH, W], F16, tag="rdiv")
            nc.scalar.dma_start(t[:], d_rdiv[:])
            late["rdiv"] = t
            t = P.tile([128, 3, 16], F16, tag="selw")
            nc.scalar.dma_start(t[:], d_selw[:])
            late["selw"] = t

        def conv1(k):
            n = SEQ[k]
            p, pad, ra = PS[n], PADS[n], RAS[n]
            rxo = 6 - 2 * pad
            wins = _windows(ra)
            nw = len(wins)
            E = pE.tile([73, RA, W], F32R, tag="E")
            nc.gpsimd.dma_start(E[72:73, 0:ra, :], d_hug[n:n + 1, 0:ra, :])
            Sbg = pS.tile([128, RA, 136], F16, tag="Sbg")
            Stg = pS.tile([72, RA, 144], F16, tag="Stg")
            nc.gpsimd.memset(Sbg[:, :, 0:4], 0.0)
            nc.gpsimd.memset(Sbg[:, :, 131:136], 0.0)
            nc.gpsimd.memset(Stg[:, :, 0:15], 0.0)
            nc.gpsimd.memset(Stg[:, :, 136:144], 0.0)
            st_etg = {}

            def denom_pair(w0):
                # denominator + softmax muls for windows w0, w0+1 (<=8 rows)
                r0 = wins[w0][0]
                rr8 = wins[w0][1] + (wins[w0 + 1][1] if w0 + 1 < nw else 0)
                rcp = pRcp.tile([72, 8, W], F32, tag="rcp")
                segs = [(0, wins[w0][1])]
                if w0 + 1 < nw:
                    segs.append((wins[w0][1], wins[w0 + 1][1]))
                for h, rr in segs:
                    den = ps_den.tile([72, 4, W], F32, tag="den")
                    nc.tensor.matmul(den[0:72, 0:rr, :], ones_l[:, 0:72],
                                     E[:, r0 + h:r0 + h + rr, :],
                                     start=True, stop=True)
                    nc.vector.reciprocal_approx_fast(rcp[0:72, h:h + rr, :],
                                                     den[0:72, 0:rr, :])
                nc.vector.tensor_mul(Sbg[0:64, r0:r0 + rr8, 3:131],
                                     E[0:64, r0:r0 + rr8, :],
                                     rcp[0:64, 0:rr8, :])
                nc.gpsimd.tensor_mul(Stg[64:72, r0:r0 + rr8, 8:136],
                                     E[64:72, r0:r0 + rr8, :],
                                     rcp[64:72, 0:rr8, :])

            for w, (r0, rr) in enumerate(wins):
                st = ps_c1bg.tile([64, 4, W], F32, tag="c1bg")
                for j in range(p):
                    nc.tensor.matmul(
                        st[0:64, 0:rr, :],
                        w1ap("bg", n, j),
                        x8["bg"][0:16 * p, r0 + rxo:r0 + rxo + rr,
                                 j + 3 - pad:j + 3 - pad + W],
                        start=(j == 0), stop=(j == p - 1))
                nc.scalar.activation(E[0:64, r0:r0 + rr, :], st[0:64, 0:rr, :],
                                     mybir.ActivationFunctionType.Exp)
                stg = ps_c1tg.tile([8, 4, W], F32, tag="c1tg")
                for j in range(p):
                    nc.tensor.matmul(
                        stg[0:8, 0:rr, :],
                        w1ap("tg", n, j),
                        x8["tg"][0:16 * p, r0 + rxo:r0 + rxo + rr,
                                 j + 3 - pad:j + 3 - pad + W],
                        start=(j == 0), stop=(j == p - 1))
                if w % 2 == 0:
                    st_etg[w // 2] = pEtg.tile([8, 8, W], F32R, tag="etg", name="etg")
                etg = st_etg[w // 2]
                h = 0 if w % 2 == 0 else wins[w - 1][1]
                nc.scalar.activation(etg[0:8, h:h + rr, :], stg[0:8, 0:rr, :],
                                     mybir.ActivationFunctionType.Exp)
                if w % 2 == 1 or w == nw - 1:
                    w0 = w - (w % 2)
                    hh = wins[w0][1] + (rr if w % 2 == 1 else 0)
                    nc.gpsimd.dma_start(
                        E[64:72, wins[w0][0]:wins[w0][0] + hh, :],
                        etg[0:8, 0:hh, :])
                    if w0 >= 2:
                        denom_pair(w0 - 2)
                # deferred loads ride behind the first windows
                if k == 0 and w == 2:
                    load_late()
            # remaining denominator pair (in-loop covered up to last-2)
            denom_pair((nw - 1) - ((nw - 1) % 2))
            state[k] = (Sbg, Stg)

        def replicas(k):
            # full-branch replica DMAs; emitted as late as possible so the
            # (conservative, queue-cumulative) sync-DMA dependency horizon of
            # earlier conv2 stages never includes them
            n = SEQ[k]
            p, ra = PS[n], RAS[n]
            Sbg, Stg = state[k]
            for g in range(p):
                nc.sync.dma_start(Stg[8 * g:8 * g + 8, 0:ra, 8 + g:136 + g],
                                  Stg[64:72, 0:ra, 8:136])
            nc.sync.dma_start(Sbg[64:128, 0:ra, 4:132], Sbg[0:64, 0:ra, 3:131])

        def fold_dma(k):
            # fold_y stage 1: align each group's rows with per-group DMAs
            # (only DMAs can shift rows per partition group); emitted right
            # after conv2(k) so the sync queue runs these before the next
            # branch's replicas.
            n = SEQ[k]
            p = PS[n]
            Rs = state[k]
            Q = {}
            for si in range(2):
                Q[si] = pQ.tile([128, BH, W], F16, tag="Q", name=f"Q{si}")
                for g in range(p):
                    nc.gpsimd.dma_start(Q[si][16 * g:16 * g + 16, :, :],
                                        Rs[si][16 * g:16 * g + 16, g:g + BH, :])
            state[("Q", k)] = Q

        def fold_mm_gen(k):
            # fold_y stage 2: contract the groups with a 0/1 selection matrix
            # (K=16p, M=16); yielded in steps so conv2 can interleave them.
            n = SEQ[k]
            p = PS[n]
            Q = state[("Q", k)]
            sel = late["selw"]
            for si in range(2):
                Qo = pQo.tile([16, BH, W], F16, tag="Qo", name=f"Qo{si}")
                for r0 in range(0, BH, 4):
                    rpf = ps_c2.tile([16, 4, W], F32, tag="c2", name="rpf")
                    nc.tensor.matmul(rpf[0:16, :, :], sel[0:16 * p, n, :],
                                     Q[si][0:16 * p, r0:r0 + 4, :],
                                     start=True, stop=True)
                    nc.scalar.activation(Qo[0:16, r0:r0 + 4, :],
                                          rpf[0:16, :, :],
                                          mybir.ActivationFunctionType.Copy)
                    yield
                nc.gpsimd.dma_start(
                    fdiv[32 * n + 16 * si:32 * n + 16 * si + 16, :, :],
                    Qo[:])
            nc.vector.scalar_tensor_tensor(
                fdiv[32 * n:32 * n + 32, :, :],
                fdiv[32 * n:32 * n + 32, :, :], 0.0,
                late["rdiv"][32 * n:32 * n + 32, :, :],
                op0=mybir.AluOpType.bypass, op1=mybir.AluOpType.mult,
                accum_out=pacc[32 * n:32 * n + 32, :])

        def conv2(k, foldgen=None):
            n = SEQ[k]
            p, pad, ra = PS[n], PADS[n], RAS[n]
            Sbg, Stg = state[k]
            w2bg, w2tg = late["w2bg"], late["w2tg"]
            Rbg = pR.tile([128, RA, W], F16, tag="R")
            Rtg = pR.tile([128, RA, W], F16, tag="R")
            nchk = (p + 1) // 2
            for r0, rr in _windows(ra):
                rp = ps_c2.tile([128, 4, W], F32, tag="c2")
                for ci in range(nchk):
                    jj = 2 * ci
                    nc.tensor.matmul(
                        rp[:, 0:rr, :],
                        w2bg[:, W2BASE[n] + ci, :],
                        Sbg[:, r0:r0 + rr, 3 + pad - jj:3 + pad - jj + W],
                        start=(ci == 0), stop=(ci == nchk - 1))
                nc.scalar.activation(Rbg[:, r0:r0 + rr, :], rp[:, 0:rr, :],
                                     mybir.ActivationFunctionType.Copy)
                rp2 = ps_c2.tile([128, 4, W], F32, tag="c2")
                nc.tensor.matmul(rp2[0:128, 0:rr, :],
                                 w2tg[0:8 * p, n, :],
                                 Stg[0:8 * p, r0:r0 + rr, 8 + pad:8 + pad + W],
                                 start=True, stop=True)
                nc.scalar.activation(Rtg[:, r0:r0 + rr, :], rp2[:, 0:rr, :],
                                     mybir.ActivationFunctionType.Copy)
                if foldgen is not None:
                    next(foldgen, None)
                    next(foldgen, None)
            if foldgen is not None:
                for _ in foldgen:
                    pass
            state[k] = (Rbg, Rtg)

        def drain(gen):
            for _ in gen:
                pass

        # ---------------- pipelined emission ----------------
        conv1(0)
        replicas(0)
        conv1(1)
        conv2(0)
        replicas(1)
        fold_dma(0)
        conv1(2)
        conv2(1, foldgen=fold_mm_gen(0))
        replicas(2)
        fold_dma(1)
        conv2(2, foldgen=fold_mm_gen(1))
        fold_dma(2)
        drain(fold_mm_gen(2))

        # deferred fdiv store: mid-kernel sync DMAs that wait on late vector
        # STTs would put those STTs inside later conv2 queue horizons
        for k in range(3):
            n = SEQ[k]
            nc.sync.dma_start(d_fdiv[32 * n:32 * n + 32, :, :],
                              fdiv[32 * n:32 * n + 32, :, :])

        # pooled partial combine -> pool_out
        pb = P.tile([32, 1], F32, tag="pb")
        pc = P.tile([32, 1], F32, tag="pc")
        nc.sync.dma_start(pb[:], pacc[32:64, :])
        nc.sync.dma_start(pc[:], pacc[64:96, :])
        pool32a = P.tile([32, 1], F32, tag="pool32a")
        pool32 = P.tile([32, 1], F32, tag="pool32")
        nc.vector.tensor_add(pool32a[:], pacc[0:32, :], pb[:])
        nc.vector.tensor_add(pool32[:], pool32a[:], pc[:])
        nc.sync.dma_start(d_pool[:], pool32[:, 0])

    nc.compile()
    return nc


def _build_B():
    nc = bacc.Bacc("TRN2", target_bir_lowering=False, debug=False,
                   num_devices=NCORES)
    d_f = nc.dram_tensor("fdiv_in", [96, BH, W], F16, kind="ExternalInput")
    d_wt = nc.dram_tensor("wt96", [96, 1], F32, kind="ExternalInput")
    d_obg = nc.dram_tensor("out_bg", [C, BH, W], F16, kind="ExternalOutput")
    d_otg = nc.dram_tensor("out_tg", [C, BH, W], F16, kind="ExternalOutput")

    with tile.TileContext(nc) as tc, ExitStack() as ctx:
        Q = ctx.enter_context(tc.tile_pool(name="q", bufs=1))
        fdv = Q.tile([96, BH, W], F16, tag="fdv")
        wt = Q.tile([96, 1], F32, tag="wt")
        gb = Q.tile([32, BH, W], F16, tag="gb")
        gc = Q.tile([32, BH, W], F16, tag="gc")
        nc.sync.dma_start(wt[:], d_wt[:])
        # row-halved software pipeline: load / scale / gather / add / store
        eng = (nc.sync, nc.scalar)
        for h in range(2):
            r = slice(16 * h, 16 * h + 16)
            eng[h].dma_start(fdv[:, r, :], d_f[:, r, :])
        for h in range(2):
            r = slice(16 * h, 16 * h + 16)
            nc.vector.tensor_scalar_mul(fdv[:, r, :], fdv[:, r, :], wt[:])
            eng[h].dma_start(gb[:, r, :], fdv[32:64, r, :])
            eng[1 - h].dma_start(gc[:, r, :], fdv[64:96, r, :])
            nc.vector.tensor_add(fdv[0:32, r, :], fdv[0:32, r, :],
                                 gb[:, r, :])
            nc.vector.tensor_add(fdv[0:32, r, :], fdv[0:32, r, :],
                                 gc[:, r, :])
            eng[h].dma_start(d_obg[:, r, :], fdv[0:16, r, :])
            eng[1 - h].dma_start(d_otg[:, r, :], fdv[16:32, r, :])

    nc.compile()
    return nc


# ======================= host-side prep =======================

def _prep_core(inputs, b, k):
    y0 = BH * k
    m = {}
    for s, key in (("bg", "bg"), ("tg", "tg")):
        x = np.asarray(inputs[key])[b]          # [C, H, W]
        x8 = np.zeros((7, C, RX, CX), np.float32)
        for g in range(7):
            lo = y0 - 6 + g
            hi = lo + RX
            slo, shi = max(lo, 0), min(hi, H)
            if slo < shi:
                x8[g, :, slo - lo:shi - lo, 3:131] = x[:, slo:shi, :]
        m[f"x8{s}"] = x8.reshape(112, RX, CX).astype(np.float16)

    hug = np.zeros((3, RA, W), np.float32)
    for n, pad in enumerate(PADS):
        for r in range(RA):
            y = y0 - pad + r
            if not (0 <= y < H):
                hug[n, r, :] = 1e30
    m["hug"] = hug

    rdiv = np.zeros((96, BH, W), np.float32)
    for n, pad in enumerate(PADS):
        yy = np.arange(H)
        rc = np.minimum(yy, pad) + np.minimum(H - 1 - yy, pad) + 1.0
        cc = np.minimum(yy[:W], pad) + np.minimum(W - 1 - yy[:W], pad) + 1.0
        div = np.outer(rc[y0:y0 + BH], cc) + 1e-8
        r = (1.0 / div).astype(np.float32)
        for si in range(2):
            base = 32 * n + 16 * si
            rdiv[base:base + 16] = r[None, :, :]
    m["rdiv"] = rdiv.astype(np.float16)

    ones = np.zeros((73, 72), np.float32)
    ones[0:64, 0:64] = 1.0
    ones[64:72, 64:72] = 1.0
    ones[72, :] = 1.0
    m["oneslhs"] = ones

    w1all = np.zeros((112, 1080), np.float32)
    w1off = {("bg", 0): 0, ("bg", 1): 192, ("bg", 2): 512,
             ("tg", 0): 960, ("tg", 1): 984, ("tg", 2): 1024}
    for s, M, nmem in (("bg", NBG, "bg_mem"), ("tg", NTG, "tg_mem")):
        for n, p in enumerate(PS):
            mem = np.asarray(inputs[f"{nmem}{n}"])          # [M, C*p*p]
            temp = float(np.asarray(inputs[f"{s}_temp{n}"])[0])
            D = C * p * p
            arr = mem.reshape(M, C, p, p)
            w1 = arr.transpose(2, 1, 3, 0).reshape(p * C, p * M)
            off = w1off[(s, n)]
            w1all[0:16 * p, off:off + p * M] = w1 * (temp / np.sqrt(D))
    m["w1all"] = w1all.astype(np.float16)

    # fold consumes group q at row shift +q where q = 2*pad - i
    w2bg = np.zeros((2, NBG, 9, 8, 16), np.float32)
    for n, p in enumerate(PS):
        pad = PADS[n]
        arr = np.asarray(inputs[f"bg_mem{n}"]).reshape(NBG, C, p, p)
        for ci in range((p + 1) // 2):
            for g in range(2):
                j = 2 * ci + g
                if j < p:
                    for i in range(p):
                        w2bg[g, :, W2BASE[n] + ci, 2 * pad - i, :] = \
                            arr[:, :, i, j]
    m["w2bg"] = w2bg.reshape(128, 9, 128).astype(np.float16)

    w2tg = np.zeros((8, NTG, 3, 8, 16), np.float32)
    for n, p in enumerate(PS):
        pad = PADS[n]
        arr = np.asarray(inputs[f"tg_mem{n}"]).reshape(NTG, C, p, p)
        for g in range(p):
            for i in range(p):
                w2tg[g, :, n, 2 * pad - i, :] = arr[:, :, i, g]
    m["w2tg"] = w2tg.reshape(64, 3, 128).astype(np.float16)

    selw = np.zeros((128, 3, 16), np.float32)
    for n, p in enumerate(PS):
        for g in range(p):
            for c in range(16):
                selw[16 * g + c, n, c] = 1.0
    m["selw"] = selw.astype(np.float16)
    return m


def _host_mlp(inputs, poolsum):
    """Per batch: pooled -> relu MLP -> softmax over scales -> wt96."""
    wt96 = np.zeros((96, 1), np.float32)
    for si, s in enumerate(("bg", "tg")):
        pooled = poolsum[16 * si:16 * si + 16] / (H * W)
        w1 = np.asarray(inputs[f"{s}_fc1_w"], np.float64)
        b1 = np.asarray(inputs[f"{s}_fc1_b"], np.float64)
        w2 = np.asarray(inputs[f"{s}_fc2_w"], np.float64)
        b2 = np.asarray(inputs[f"{s}_fc2_b"], np.float64)
        hdn = np.maximum(w1 @ pooled + b1, 0.0)
        logits = (w2 @ hdn + b2).reshape(3, 16)
        e = np.exp(logits - logits.max(axis=0, keepdims=True))
        wt = e / e.sum(axis=0, keepdims=True)
        for n in range(3):
            wt96[32 * n + 16 * si:32 * n + 16 * si + 16, 0] = wt[n]
    return wt96


def kernel(**inputs):
    if "ncA" not in _CACHE:
        _CACHE["ncA"] = _build_A()
        _CACHE["ncB"] = _build_B()

    in_maps = []
    for core in range(NCORES):
        b, k = divmod(core, NBANDS)
        in_maps.append(_prep_core(inputs, b, k))

    resA = run_bass_kernel_spmd(_CACHE["ncA"], in_maps, list(range(NCORES)))

    # host glue: reduce pooled partials within each batch's 4-band group,
    # then the tiny fusion MLP (exact, fp64)
    wt = {}
    for b in range(B):
        poolsum = np.sum([resA.results[b * NBANDS + k]["pool_out"]
                          for k in range(NBANDS)], axis=0).astype(np.float64)
        wt[b] = _host_mlp(inputs, poolsum)

    in_mapsB = []
    for core in range(NCORES):
        b, k = divmod(core, NBANDS)
        in_mapsB.append({
            "fdiv_in": resA.results[core]["fdiv_out"],
            "wt96": wt[b],
        })
    resB = run_bass_kernel_spmd(_CACHE["ncB"], in_mapsB, list(range(NCORES)))

    f_bc = np.zeros((B, C, H, W), np.float32)
    f_tg = np.zeros((B, C, H, W), np.float32)
    for core in range(NCORES):
        b, k = divmod(core, NBANDS)
        y0 = BH * k
        f_bc[b, :, y0:y0 + BH, :] = resB.results[core]["out_bg"].astype(np.float32)
        f_tg[b, :, y0:y0 + BH, :] = resB.results[core]["out_tg"].astype(np.float32)
    return (f_bc, f_tg)



# revision 9
# speedup vs baseline: 1.0805x; 1.0805x over previous
"""DualMemorySystem Trainium2 kernel — 8-core SPMD (batch x 4 row-bands).

Per core: one (b, 32-row out band). Convolution form of unfold/attention/fold:
  sim = conv(x, mem)      -> p matmuls per 4-row window, K=(kernel-row, c), fp16.
                             tg stream (M=8) rides CONCURRENTLY in PE column
                             strip 2 via tile_position=(0,64), sharing the bg
                             window's PSUM bank at partitions 64:72 — proven
                             correct+concurrent on HW (probe1).
  att = softmax_m(sim)    -> ONE exp (ACT) covering bg+tg [0:72] -> E bf16,
                             ones-matmul partition sum, reciprocal (DVE bf16),
                             multiplies (DVE bg + GpSimd tg).
  R_i = conv_x(att, mem)  -> matmuls over col-shifted att replicas, fp16.
  out = fold_y(R)         -> row-aligned Q copies (gpsimd DMA), then per 4-row
                             chunk: bg sel-matmul -> PSUM[0:16] and tg
                             sel-matmul -> PSUM[32:48] (tile_position (0,32)),
                             drained by ONE fused STT: out = PSUM * rdiv,
                             accum_out -> pooled partial. No Qo copies.
  fusion: pooled partials -> host MLP between launches -> tiny phase-B kernel
          applies softmax weights + combines.

Software-pipelined emission: branch order (p=7, p=5, p=3); per branch the
PE stream is conv1(k) [denominator matmuls trail at a 2-window lag], then
conv2(k-1), so the PE never waits on the current branch's softmax chain.
Sbg replicas split into row chunks issued right after the muls that produce
them, on alternating DMA queues (sync / scalar / sync per branch) so no conv2
stage's queue-cumulative horizon includes a later branch's replicas.

Hardware constraints baked in (probed): matmul dst partition base must equal
tile_position[1] (0 unless col-tiled); engines cannot remap partitions (only
DMA/PE move data across partitions); no divide ALU on DVE; DVE ops need
32-aligned partition bases; only gpsimd DMAs may cast dtypes; DMA issue costs
~0.6us of issuing-engine time regardless of size (so batch DMAs); PSUM is 8
banks x 2KB/partition; concurrent col-tiled matmuls need disjoint 32-col
strips (moving data streams via separate XBUSes).
"""
import numpy as np
from contextlib import ExitStack

import concourse.bass as bass
import concourse.bacc as bacc
import concourse.tile as tile
from concourse import mybir
from concourse.bass_utils import run_bass_kernel_spmd

F32 = mybir.dt.float32
BF16 = mybir.dt.bfloat16
F16 = mybir.dt.float16

B, C, H, W = 2, 16, 128, 128
PS = (3, 5, 7)
PADS = (1, 2, 3)
NBG, NTG = 64, 8
NCORES = 8
NBANDS = 4
BH = H // NBANDS            # 32 out rows per core
RX = 38                     # x replica rows per core
CX = 134                    # x cols with halo (128 + 6)
RA = 38                     # max att rows (32 + 2*padmax)
RAS = [BH + 2 * p for p in PADS]   # att rows per branch: 34, 36, 38
SEQ = (2, 1, 0)             # branch processing order: p=7, 5, 3
W2BASE = [0, 2, 5]

_CACHE = {}
C1SHARED = True


def _windows(ra):
    return [(r0, min(4, ra - r0)) for r0 in range(0, ra, 4)]


def _build_A():
    nc = bacc.Bacc("TRN2", target_bir_lowering=False, debug=False,
                   num_devices=NCORES)

    d_x8bg = nc.dram_tensor("x8bg", [112, RX, CX], F16, kind="ExternalInput")
    d_x8tg = nc.dram_tensor("x8tg", [112, RX, CX], F16, kind="ExternalInput")
    d_hug = nc.dram_tensor("hug", [3, RA, W], F32, kind="ExternalInput")
    d_rdiv = nc.dram_tensor("rdiv", [48, 3, BH, W], F16, kind="ExternalInput")
    d_ones = nc.dram_tensor("oneslhs", [73, 72], BF16, kind="ExternalInput")
    d_w1all = nc.dram_tensor("w1all", [112, 1080], F16, kind="ExternalInput")
    d_selw = nc.dram_tensor("selw", [128, 3, 48], F16, kind="ExternalInput")
    d_w2bg = nc.dram_tensor("w2bg", [128, 9, 128], F16, kind="ExternalInput")
    d_w2tg = nc.dram_tensor("w2tg", [64, 3, 128], F16, kind="ExternalInput")
    d_fdiv = nc.dram_tensor("fdiv_out", [96, BH, W], F16,
                            kind="ExternalOutput")
    d_pool = nc.dram_tensor("pool_out", [48, 24], F32, kind="ExternalOutput")

    with tile.TileContext(nc) as tc, ExitStack() as ctx:
        P = ctx.enter_context(tc.tile_pool(name="persist", bufs=1))
        pE = ctx.enter_context(tc.tile_pool(name="epool", bufs=2))
        pRcp = ctx.enter_context(tc.tile_pool(name="rcp", bufs=2))
        pS = ctx.enter_context(tc.tile_pool(name="spool", bufs=2))
        pR = ctx.enter_context(tc.tile_pool(name="rsb", bufs=4))
        pQ = ctx.enter_context(tc.tile_pool(name="qpool", bufs=2))
        ps_c1 = ctx.enter_context(
            tc.tile_pool(name="pc1", bufs=2, space=bass.MemorySpace.PSUM))
        ps_c1t = ctx.enter_context(
            tc.tile_pool(name="pc1t", bufs=2, space=bass.MemorySpace.PSUM))
        ps_c2 = ctx.enter_context(
            tc.tile_pool(name="pc2", bufs=2, space=bass.MemorySpace.PSUM))
        ps_f = ctx.enter_context(
            tc.tile_pool(name="pf", bufs=2, space=bass.MemorySpace.PSUM))

        # ---- startup-critical loads only: x8 images + first-branch weights
        x8 = {}
        w1all = P.tile([112, 1080], F16, tag="w1all")
        nc.scalar.dma_start(w1all[:, 512:960], d_w1all[:, 512:960])
        nc.scalar.dma_start(w1all[:, 1024:1080], d_w1all[:, 1024:1080])
        t = P.tile([112, RX, CX], F16, tag="x8bg")
        nc.sync.dma_start(t[:, 0:20, :], d_x8bg[:, 0:20, :])
        x8["bg"] = t
        t = P.tile([112, RX, CX], F16, tag="x8tg")
        nc.gpsimd.dma_start(t[:, 0:20, :], d_x8tg[:, 0:20, :])
        x8["tg"] = t
        nc.sync.dma_start(x8["bg"][:, 20:RX, :], d_x8bg[:, 20:RX, :])
        nc.gpsimd.dma_start(x8["tg"][:, 20:RX, :], d_x8tg[:, 20:RX, :])
        nc.scalar.dma_start(w1all[:, 0:512], d_w1all[:, 0:512])
        nc.scalar.dma_start(w1all[:, 960:1024], d_w1all[:, 960:1024])
        # packed col offsets: bg0,bg1,bg2 then tg0,tg1,tg2
        W1OFF = {("bg", 0): 0, ("bg", 1): 192, ("bg", 2): 512,
                 ("tg", 0): 960, ("tg", 1): 984, ("tg", 2): 1024}

        def w1ap(s, n, j):
            M = NBG if s == "bg" else NTG
            off = W1OFF[(s, n)] + j * M
            return w1all[0:16 * PS[n], off:off + M]

        ones_l = P.tile([73, 72], BF16, tag="ones")
        nc.gpsimd.dma_start(ones_l[:], d_ones[:])

        # per-branch fused fold outputs: [0:16]=bg, [16:32]=zeros, [32:48]=tg
        fdiv48 = [P.tile([48, BH, W], F16, tag=f"fdiv{n}", name=f"fdiv{n}")
                  for n in range(3)]
        pacc = P.tile([48, 24], F32, tag="pacc")
        late = {}   # deferred persistent tiles: w2bg, w2tg, rdiv48, selw

        state = {}

        def load_late():
            t = P.tile([128, 9, 128], F16, tag="w2bg")
            nc.scalar.dma_start(t[:], d_w2bg[:])
            late["w2bg"] = t
            t = P.tile([64, 3, 128], F16, tag="w2tg")
            nc.scalar.dma_start(t[:], d_w2tg[:])
            late["w2tg"] = t
            t = P.tile([48, 3, BH, W], F16, tag="rdiv")
            nc.scalar.dma_start(t[:], d_rdiv[:])
            late["rdiv"] = t
            t = P.tile([128, 3, 48], F16, tag="selw")
            nc.scalar.dma_start(t[:], d_selw[:])
            late["selw"] = t

        # Replica DMA queue choice per branch avoids queue-cumulative
        # horizon stalls: a conv2 stage must never (transitively) wait on a
        # LATER branch's replica DMAs. Branch 1's replicas are emitted whole
        # after conv2(0); branches 0/2 emit row chunks inline with the muls
        # that produce them — branch 2 on the vector queue, which no earlier
        # conv2 stage depends on.
        REPQ = {0: "sync", 1: None, 2: "gpsimd"}

        def conv1(k):
            n = SEQ[k]
            p, pad, ra = PS[n], PADS[n], RAS[n]
            rxo = 6 - 2 * pad
            wins = _windows(ra)
            nw = len(wins)
            E = pE.tile([73, RA, W], BF16, tag="E")
            nc.gpsimd.dma_start(E[72:73, 0:ra, :], d_hug[n:n + 1, 0:ra, :])
            Sbg = pS.tile([128, RA, 136], F16, tag="Sbg")
            Stg = pS.tile([72, RA, 144], F16, tag="Stg")
            nc.gpsimd.memset(Sbg[:, :, 0:4], 0.0)
            nc.gpsimd.memset(Sbg[:, :, 131:136], 0.0)
            nc.gpsimd.memset(Stg[:, :, 0:15], 0.0)
            nc.gpsimd.memset(Stg[:, :, 136:144], 0.0)
            repq = getattr(nc, REPQ[k]) if REPQ[k] else None

            def denom_pair(w0):
                # denominator + softmax muls for windows w0, w0+1 (<=8 rows)
                r0 = wins[w0][0]
                rr8 = wins[w0][1] + (wins[w0 + 1][1] if w0 + 1 < nw else 0)
                rcp = pRcp.tile([72, 8, W], F32, tag="rcp")
                segs = [(0, wins[w0][1])]
                if w0 + 1 < nw:
                    segs.append((wins[w0][1], wins[w0 + 1][1]))
                for h, rr in segs:
                    den = ps_c1t.tile([72, 4, W], F32, tag="c1t", name="den")
                    nc.tensor.matmul(den[0:72, 0:rr, :], ones_l[:, 0:72],
                                     E[:, r0 + h:r0 + h + rr, :],
                                     start=True, stop=True)
                    nc.vector.reciprocal_approx_fast(rcp[0:72, h:h + rr, :],
                                                     den[0:72, 0:rr, :])
                nc.vector.tensor_mul(Sbg[0:64, r0:r0 + rr8, 3:131],
                                     E[0:64, r0:r0 + rr8, :],
                                     rcp[0:64, 0:rr8, :])
                nc.gpsimd.tensor_mul(Stg[64:72, r0:r0 + rr8, 8:136],
                                     E[64:72, r0:r0 + rr8, :],
                                     rcp[64:72, 0:rr8, :])
                # bg replica chunk for exactly these rows rides immediately
                if repq is not None:
                    repq.dma_start(Sbg[64:128, r0:r0 + rr8, 4:132],
                                   Sbg[0:64, r0:r0 + rr8, 3:131])

            for w, (r0, rr) in enumerate(wins):
                # blocked emission (interleaving tile-positions stalls the PE
                # pipeline — probes A/D); tg goes to partitions 64:72 via
                # tile_position so its exp writes E[64:72] directly (no
                # cross-partition DMA). C1SHARED: one PSUM bank for both
                # groups sequentially -> one merged exp.
                if C1SHARED:
                    stB = ps_c1.tile([72, 4, W], F32, tag="c1")
                    stT = stB
                else:
                    stB = ps_c1.tile([64, 4, W], F32, tag="c1")
                    stT = ps_c1t.tile([72, 4, W], F32, tag="c1t", name="stT")
                for j in range(p):
                    nc.tensor.matmul(
                        stB[0:64, 0:rr, :],
                        w1ap("bg", n, j),
                        x8["bg"][0:16 * p, r0 + rxo:r0 + rxo + rr,
                                 j + 3 - pad:j + 3 - pad + W],
                        start=(j == 0), stop=(j == p - 1),
                        skip_group_check=True)
                for j in range(p):
                    nc.tensor.matmul(
                        stT[64:72, 0:rr, :],
                        w1ap("tg", n, j),
                        x8["tg"][0:16 * p, r0 + rxo:r0 + rxo + rr,
                                 j + 3 - pad:j + 3 - pad + W],
                        start=(j == 0), stop=(j == p - 1),
                        tile_position=(0, 64),
                        skip_group_check=True)
                if C1SHARED:
                    nc.scalar.activation(E[0:72, r0:r0 + rr, :],
                                         stB[0:72, 0:rr, :],
                                         mybir.ActivationFunctionType.Exp)
                else:
                    nc.scalar.activation(E[0:64, r0:r0 + rr, :],
                                         stB[0:64, 0:rr, :],
                                         mybir.ActivationFunctionType.Exp)
                    nc.scalar.activation(E[64:72, r0:r0 + rr, :],
                                         stT[64:72, 0:rr, :],
                                         mybir.ActivationFunctionType.Exp)
                if (w % 2 == 1 or w == nw - 1) and w >= 3:
                    denom_pair(w - 2 - (w % 2))
                # deferred loads ride behind the first windows
                if k == 0 and w == 2:
                    load_late()
            # remaining denominator pair
            denom_pair((nw - 1) - ((nw - 1) % 2))
            state[k] = (Sbg, Stg)

        def replicas_tg(k):
            # tg replica DMAs (whole branch, p col-shifted copies)
            n = SEQ[k]
            p, ra = PS[n], RAS[n]
            _, Stg = state[k]
            repq = getattr(nc, REPQ[k] or "sync")
            for g in range(p):
                repq.dma_start(Stg[8 * g:8 * g + 8, 0:ra, 8 + g:136 + g],
                               Stg[64:72, 0:ra, 8:136])

        def replicas_bg(k):
            # whole-branch bg replica (branch 1: emitted after conv2(0) so
            # conv2(0)'s sync horizon never includes it)
            n = SEQ[k]
            ra = RAS[n]
            Sbg, _ = state[k]
            nc.sync.dma_start(Sbg[64:128, 0:ra, 4:132],
                              Sbg[0:64, 0:ra, 3:131])

        def fold_dma(k):
            # fold_y stage 1: align each group's rows with per-group DMAs
            # (only DMAs can shift rows per partition group); emitted right
            # after conv2(k) so the gpsimd queue runs these before the next
            # branch's fold needs the data.
            n = SEQ[k]
            p = PS[n]
            Rs = state[k]
            Q = {}
            for si in range(2):
                Q[si] = pQ.tile([128, BH, W], F16, tag="Q", name=f"Q{si}")
                for g in range(p):
                    nc.gpsimd.dma_start(Q[si][16 * g:16 * g + 16, :, :],
                                        Rs[si][16 * g:16 * g + 16, g:g + BH, :])
            state[("Q", k)] = Q

        def fold_mm_gen(k):
            # fold_y stage 2: contract the groups with a 0/1 selection matrix.
            # bg -> PSUM[0:16] (tile 0,0), tg -> PSUM[32:48] (tile 0,32), both
            # in ONE bank; drained by one fused STT (x rdiv, accum -> pool).
            # PSUM rows 16:32 are stale-but-finite; rdiv is 0 there.
            n = SEQ[k]
            p = PS[n]
            Q = state[("Q", k)]
            sel = late["selw"]
            rdiv = late["rdiv"]
            for ci, r0 in enumerate(range(0, BH, 4)):
                rpf = ps_f.tile([48, 4, W], F32, tag="f")
                nc.tensor.matmul(rpf[0:48, :, :], sel[0:16 * p, n, 0:48],
                                 Q[0][0:16 * p, r0:r0 + 4, :],
                                 start=True, stop=True, skip_group_check=True)
                nc.tensor.matmul(rpf[32:48, :, :], sel[0:16 * p, n, 0:16],
                                 Q[1][0:16 * p, r0:r0 + 4, :],
                                 start=True, stop=True,
                                 tile_position=(0, 32), skip_group_check=True)
                # gpsimd cannot read PSUM -> all fused drains on vector
                nc.vector.scalar_tensor_tensor(
                    fdiv48[n][:, r0:r0 + 4, :],
                    rpf[:, :, :], 0.0,
                    rdiv[:, n, r0:r0 + 4, :],
                    op0=mybir.AluOpType.bypass, op1=mybir.AluOpType.mult,
                    accum_out=pacc[:, 8 * n + ci:8 * n + ci + 1])
                yield

        def conv2(k, foldgen=None):
            n = SEQ[k]
            p, pad, ra = PS[n], PADS[n], RAS[n]
            Sbg, Stg = state[k]
            w2bg, w2tg = late["w2bg"], late["w2tg"]
            Rbg = pR.tile([128, RA, W], F16, tag="R")
            Rtg = pR.tile([128, RA, W], F16, tag="R")
            nchk = (p + 1) // 2
            for r0, rr in _windows(ra):
                rp = ps_c2.tile([128, 4, W], F32, tag="c2")
                for ci in range(nchk):
                    jj = 2 * ci
                    nc.tensor.matmul(
                        rp[:, 0:rr, :],
                        w2bg[:, W2BASE[n] + ci, :],
                        Sbg[:, r0:r0 + rr, 3 + pad - jj:3 + pad - jj + W],
                        start=(ci == 0), stop=(ci == nchk - 1))
                nc.scalar.activation(Rbg[:, r0:r0 + rr, :], rp[:, 0:rr, :],
                                     mybir.ActivationFunctionType.Copy)
                rp2 = ps_c2.tile([128, 4, W], F32, tag="c2")
                nc.tensor.matmul(rp2[0:128, 0:rr, :],
                                 w2tg[0:8 * p, n, :],
                                 Stg[0:8 * p, r0:r0 + rr, 8 + pad:8 + pad + W],
                                 start=True, stop=True)
                if (r0 // 4) % 2 == 0:
                    nc.scalar.activation(Rtg[:, r0:r0 + rr, :], rp2[:, 0:rr, :],
                                         mybir.ActivationFunctionType.Copy)
                else:
                    nc.vector.tensor_copy(Rtg[:, r0:r0 + rr, :],
                                          rp2[:, 0:rr, :])
                if foldgen is not None:
                    next(foldgen, None)
            if foldgen is not None:
                for _ in foldgen:
                    pass
            state[k] = (Rbg, Rtg)

        def drain(gen):
            for _ in gen:
                pass

        # ---------------- pipelined emission ----------------
        conv1(0)
        replicas_tg(0)
        conv1(1)
        conv2(0)
        replicas_bg(1)
        replicas_tg(1)
        fold_dma(0)
        conv1(2)
        replicas_tg(2)
        conv2(1, foldgen=fold_mm_gen(0))
        fold_dma(1)
        conv2(2, foldgen=fold_mm_gen(1))
        fold_dma(2)
        drain(fold_mm_gen(2))

        # deferred fdiv stores (bg piece + tg piece per branch)
        for k in range(3):
            n = SEQ[k]
            nc.sync.dma_start(d_fdiv[32 * n:32 * n + 16, :, :],
                              fdiv48[n][0:16, :, :])
            nc.sync.dma_start(d_fdiv[32 * n + 16:32 * n + 32, :, :],
                              fdiv48[n][32:48, :, :])
        nc.sync.dma_start(d_pool[:], pacc[:])

    nc.compile()
    return nc


def _build_B():
    nc = bacc.Bacc("TRN2", target_bir_lowering=False, debug=False,
                   num_devices=NCORES)
    d_f = nc.dram_tensor("fdiv_in", [96, BH, W], F16, kind="ExternalInput")
    d_wt = nc.dram_tensor("wt96", [96, 1], F32, kind="ExternalInput")
    d_obg = nc.dram_tensor("out_bg", [C, BH, W], F16, kind="ExternalOutput")
    d_otg = nc.dram_tensor("out_tg", [C, BH, W], F16, kind="ExternalOutput")

    with tile.TileContext(nc) as tc, ExitStack() as ctx:
        Q = ctx.enter_context(tc.tile_pool(name="q", bufs=1))
        fdv = Q.tile([96, BH, W], F16, tag="fdv")
        wt = Q.tile([96, 1], F32, tag="wt")
        gb = Q.tile([32, BH, W], F16, tag="gb")
        gc = Q.tile([32, BH, W], F16, tag="gc")
        nc.sync.dma_start(wt[:], d_wt[:])
        # row-halved software pipeline: load / scale / gather / add / store
        eng = (nc.sync, nc.scalar)
        for h in range(2):
            r = slice(16 * h, 16 * h + 16)
            eng[h].dma_start(fdv[:, r, :], d_f[:, r, :])
        for h in range(2):
            r = slice(16 * h, 16 * h + 16)
            nc.vector.tensor_scalar_mul(fdv[:, r, :], fdv[:, r, :], wt[:])
            eng[h].dma_start(gb[:, r, :], fdv[32:64, r, :])
            eng[1 - h].dma_start(gc[:, r, :], fdv[64:96, r, :])
            nc.vector.tensor_add(fdv[0:32, r, :], fdv[0:32, r, :],
                                 gb[:, r, :])
            nc.vector.tensor_add(fdv[0:32, r, :], fdv[0:32, r, :],
                                 gc[:, r, :])
            eng[h].dma_start(d_obg[:, r, :], fdv[0:16, r, :])
            eng[1 - h].dma_start(d_otg[:, r, :], fdv[16:32, r, :])

    nc.compile()
    return nc


# ======================= host-side prep =======================

def _prep_core(inputs, b, k):
    y0 = BH * k
    m = {}
    for s, key in (("bg", "bg"), ("tg", "tg")):
        x = np.asarray(inputs[key])[b]          # [C, H, W]
        x8 = np.zeros((7, C, RX, CX), np.float32)
        for g in range(7):
            lo = y0 - 6 + g
            hi = lo + RX
            slo, shi = max(lo, 0), min(hi, H)
            if slo < shi:
                x8[g, :, slo - lo:shi - lo, 3:131] = x[:, slo:shi, :]
        m[f"x8{s}"] = x8.reshape(112, RX, CX).astype(np.float16)

    hug = np.zeros((3, RA, W), np.float32)
    for n, pad in enumerate(PADS):
        for r in range(RA):
            y = y0 - pad + r
            if not (0 <= y < H):
                hug[n, r, :] = 1e30
    m["hug"] = hug

    # fused fold drain: [0:16]=bg rdiv, [16:32]=0 (kills stale PSUM), [32:48]=tg
    rdiv = np.zeros((48, 3, BH, W), np.float32)
    for n, pad in enumerate(PADS):
        yy = np.arange(H)
        rc = np.minimum(yy, pad) + np.minimum(H - 1 - yy, pad) + 1.0
        cc = np.minimum(yy[:W], pad) + np.minimum(W - 1 - yy[:W], pad) + 1.0
        div = np.outer(rc[y0:y0 + BH], cc) + 1e-8
        r = (1.0 / div).astype(np.float32)
        rdiv[0:16, n] = r[None, :, :]
        rdiv[32:48, n] = r[None, :, :]
    m["rdiv"] = rdiv.astype(np.float16)

    ones = np.zeros((73, 72), np.float32)
    ones[0:64, 0:64] = 1.0
    ones[64:72, 64:72] = 1.0
    ones[72, :] = 1.0
    m["oneslhs"] = ones        # cast to bf16 at upload below

    w1all = np.zeros((112, 1080), np.float32)
    w1off = {("bg", 0): 0, ("bg", 1): 192, ("bg", 2): 512,
             ("tg", 0): 960, ("tg", 1): 984, ("tg", 2): 1024}
    for s, M, nmem in (("bg", NBG, "bg_mem"), ("tg", NTG, "tg_mem")):
        for n, p in enumerate(PS):
            mem = np.asarray(inputs[f"{nmem}{n}"])          # [M, C*p*p]
            temp = float(np.asarray(inputs[f"{s}_temp{n}"])[0])
            D = C * p * p
            arr = mem.reshape(M, C, p, p)
            w1 = arr.transpose(2, 1, 3, 0).reshape(p * C, p * M)
            off = w1off[(s, n)]
            w1all[0:16 * p, off:off + p * M] = w1 * (temp / np.sqrt(D))
    m["w1all"] = w1all.astype(np.float16)

    # fold consumes group q at row shift +q where q = 2*pad - i
    w2bg = np.zeros((2, NBG, 9, 8, 16), np.float32)
    for n, p in enumerate(PS):
        pad = PADS[n]
        arr = np.asarray(inputs[f"bg_mem{n}"]).reshape(NBG, C, p, p)
        for ci in range((p + 1) // 2):
            for g in range(2):
                j = 2 * ci + g
                if j < p:
                    for i in range(p):
                        w2bg[g, :, W2BASE[n] + ci, 2 * pad - i, :] = \
                            arr[:, :, i, j]
    m["w2bg"] = w2bg.reshape(128, 9, 128).astype(np.float16)

    w2tg = np.zeros((8, NTG, 3, 8, 16), np.float32)
    for n, p in enumerate(PS):
        pad = PADS[n]
        arr = np.asarray(inputs[f"tg_mem{n}"]).reshape(NTG, C, p, p)
        for g in range(p):
            for i in range(p):
                w2tg[g, :, n, 2 * pad - i, :] = arr[:, :, i, g]
    m["w2tg"] = w2tg.reshape(64, 3, 128).astype(np.float16)

    selw = np.zeros((128, 3, 48), np.float32)
    for n, p in enumerate(PS):
        for g in range(p):
            for c in range(16):
                selw[16 * g + c, n, c] = 1.0
    m["selw"] = selw.astype(np.float16)

    # bf16 upload for the ones matrix
    import ml_dtypes
    m["oneslhs"] = m["oneslhs"].astype(ml_dtypes.bfloat16)
    return m


def _host_mlp(inputs, poolsum):
    """Per batch: pooled -> relu MLP -> softmax over scales -> wt96.

    poolsum: [48, 24] summed over band cores; rows 0:16 = bg per-channel
    partial sums per (branch, chunk) col = 8n+chunk; rows 32:48 = tg.
    """
    wt96 = np.zeros((96, 1), np.float32)
    for si, s in enumerate(("bg", "tg")):
        rows = slice(0, 16) if si == 0 else slice(32, 48)
        pooled = poolsum[rows, :].sum(axis=1) / (H * W)
        w1 = np.asarray(inputs[f"{s}_fc1_w"], np.float64)
        b1 = np.asarray(inputs[f"{s}_fc1_b"], np.float64)
        w2 = np.asarray(inputs[f"{s}_fc2_w"], np.float64)
        b2 = np.asarray(inputs[f"{s}_fc2_b"], np.float64)
        hdn = np.maximum(w1 @ pooled + b1, 0.0)
        logits = (w2 @ hdn + b2).reshape(3, 16)
        e = np.exp(logits - logits.max(axis=0, keepdims=True))
        wt = e / e.sum(axis=0, keepdims=True)
        for n in range(3):
            wt96[32 * n + 16 * si:32 * n + 16 * si + 16, 0] = wt[n]
    return wt96


def kernel(**inputs):
    if "ncA" not in _CACHE:
        _CACHE["ncA"] = _build_A()
        _CACHE["ncB"] = _build_B()

    in_maps = []
    for core in range(NCORES):
        b, k = divmod(core, NBANDS)
        in_maps.append(_prep_core(inputs, b, k))

    resA = run_bass_kernel_spmd(_CACHE["ncA"], in_maps, list(range(NCORES)))

    # host glue: reduce pooled partials within each batch's 4-band group,
    # then the tiny fusion MLP (exact, fp64)
    wt = {}
    for b in range(B):
        poolsum = np.sum([resA.results[b * NBANDS + k]["pool_out"]
                          for k in range(NBANDS)], axis=0).astype(np.float64)
        wt[b] = _host_mlp(inputs, poolsum)

    in_mapsB = []
    for core in range(NCORES):
        b, k = divmod(core, NBANDS)
        in_mapsB.append({
            "fdiv_in": resA.results[core]["fdiv_out"],
            "wt96": wt[b],
        })
    resB = run_bass_kernel_spmd(_CACHE["ncB"], in_mapsB, list(range(NCORES)))

    f_bc = np.zeros((B, C, H, W), np.float32)
    f_tg = np.zeros((B, C, H, W), np.float32)
    for core in range(NCORES):
        b, k = divmod(core, NBANDS)
        y0 = BH * k
        f_bc[b, :, y0:y0 + BH, :] = resB.results[core]["out_bg"].astype(np.float32)
        f_tg[b, :, y0:y0 + BH, :] = resB.results[core]["out_tg"].astype(np.float32)
    return (f_bc, f_tg)


# revision 10
# speedup vs baseline: 1.1891x; 1.1004x over previous
"""DualMemorySystem Trainium2 kernel — 8-core SPMD (batch x 4 row-bands).

Per core: one (b, 32-row out band). Convolution form of unfold/attention/fold:
  sim = conv(x, mem)      -> p matmuls per 4-row window, K=(kernel-row, c), fp16.
                             tg stream (M=8) rides CONCURRENTLY in PE column
                             strip 2 via tile_position=(0,64), sharing the bg
                             window's PSUM bank at partitions 64:72 — proven
                             correct+concurrent on HW (probe1).
  att = softmax_m(sim)    -> ONE exp (ACT) covering bg+tg [0:72] -> E bf16,
                             ones-matmul partition sum, reciprocal (DVE bf16),
                             multiplies (DVE bg + GpSimd tg).
  R_i = conv_x(att, mem)  -> matmuls over col-shifted att replicas, fp16.
  out = fold_y(R)         -> row-aligned Q copies (gpsimd DMA), then per 4-row
                             chunk: bg sel-matmul -> PSUM[0:16] and tg
                             sel-matmul -> PSUM[32:48] (tile_position (0,32)),
                             drained by ONE fused STT: out = PSUM * rdiv,
                             accum_out -> pooled partial. No Qo copies.
  fusion: pooled partials -> host MLP between launches -> tiny phase-B kernel
          applies softmax weights + combines.

Software-pipelined emission: branch order (p=7, p=5, p=3); per branch the
PE stream is conv1(k) [denominator matmuls trail at a 2-window lag], then
conv2(k-1), so the PE never waits on the current branch's softmax chain.
Sbg replicas split into row chunks issued right after the muls that produce
them, on alternating DMA queues (sync / scalar / sync per branch) so no conv2
stage's queue-cumulative horizon includes a later branch's replicas.

Hardware constraints baked in (probed): matmul dst partition base must equal
tile_position[1] (0 unless col-tiled); engines cannot remap partitions (only
DMA/PE move data across partitions); no divide ALU on DVE; DVE ops need
32-aligned partition bases; only gpsimd DMAs may cast dtypes; DMA issue costs
~0.6us of issuing-engine time regardless of size (so batch DMAs); PSUM is 8
banks x 2KB/partition; concurrent col-tiled matmuls need disjoint 32-col
strips (moving data streams via separate XBUSes).
"""
import numpy as np
from contextlib import ExitStack

import concourse.bass as bass
import concourse.bacc as bacc
import concourse.tile as tile
from concourse import mybir
from concourse.bass_utils import run_bass_kernel_spmd

F32 = mybir.dt.float32
BF16 = mybir.dt.bfloat16
F16 = mybir.dt.float16

B, C, H, W = 2, 16, 128, 128
PS = (3, 5, 7)
PADS = (1, 2, 3)
NBG, NTG = 64, 8
NCORES = 8
NBANDS = 4
BH = H // NBANDS            # 32 out rows per core
RX = 38                     # x replica rows per core
CX = 134                    # x cols with halo (128 + 6)
RA = 38                     # max att rows (32 + 2*padmax)
RAS = [BH + 2 * p for p in PADS]   # att rows per branch: 34, 36, 38
SEQ = (2, 1, 0)             # branch processing order: p=7, 5, 3
W2BASE = [0, 2, 5]

_CACHE = {}
C1SHARED = True


def _windows(ra):
    return [(r0, min(4, ra - r0)) for r0 in range(0, ra, 4)]


def _build_A():
    nc = bacc.Bacc("TRN2", target_bir_lowering=False, debug=False,
                   num_devices=NCORES)

    d_x8bg = nc.dram_tensor("x8bg", [112, RX, CX], F16, kind="ExternalInput")
    d_x8tg = nc.dram_tensor("x8tg", [112, RX, CX], F16, kind="ExternalInput")
    d_hug = nc.dram_tensor("hug", [3, RA, W], F32, kind="ExternalInput")
    d_rdiv = nc.dram_tensor("rdiv", [48, 3, BH, W], F16, kind="ExternalInput")
    d_ones = nc.dram_tensor("oneslhs", [73, 72], BF16, kind="ExternalInput")
    d_w1all = nc.dram_tensor("w1all", [112, 1080], F16, kind="ExternalInput")
    d_selw = nc.dram_tensor("selw", [128, 3, 48], F16, kind="ExternalInput")
    d_w2bg = nc.dram_tensor("w2bg", [128, 9, 128], F16, kind="ExternalInput")
    d_w2tg = nc.dram_tensor("w2tg", [64, 3, 128], F16, kind="ExternalInput")
    d_fdiv = nc.dram_tensor("fdiv_out", [96, BH, W], F16,
                            kind="ExternalOutput")
    d_pool = nc.dram_tensor("pool_out", [48, 24], F32, kind="ExternalOutput")

    with tile.TileContext(nc) as tc, ExitStack() as ctx:
        P = ctx.enter_context(tc.tile_pool(name="persist", bufs=1))
        pE = ctx.enter_context(tc.tile_pool(name="epool", bufs=2))
        pRcp = ctx.enter_context(tc.tile_pool(name="rcp", bufs=2))
        pS = ctx.enter_context(tc.tile_pool(name="spool", bufs=2))
        pR = ctx.enter_context(tc.tile_pool(name="rsb", bufs=4))
        pQ = ctx.enter_context(tc.tile_pool(name="qpool", bufs=2))
        # two PSUM pools only: conv1 windows (4-deep) and a shared
        # conv2/fold/den pool (4-deep) so PSUM-bank reuse never couples the
        # PE to a drain that ran fewer than ~3 instructions ago
        ps_c1 = ctx.enter_context(
            tc.tile_pool(name="pc1", bufs=4, space=bass.MemorySpace.PSUM))
        ps_c2 = ctx.enter_context(
            tc.tile_pool(name="pc2", bufs=4, space=bass.MemorySpace.PSUM))

        # ---- startup-critical loads only: x8 images + first-branch weights
        x8 = {}
        w1all = P.tile([112, 1080], F16, tag="w1all")
        nc.scalar.dma_start(w1all[:, 512:960], d_w1all[:, 512:960])
        nc.scalar.dma_start(w1all[:, 1024:1080], d_w1all[:, 1024:1080])
        t = P.tile([112, RX, CX], F16, tag="x8bg")
        nc.sync.dma_start(t[:, 0:20, :], d_x8bg[:, 0:20, :])
        x8["bg"] = t
        t = P.tile([112, RX, CX], F16, tag="x8tg")
        nc.gpsimd.dma_start(t[:, 0:20, :], d_x8tg[:, 0:20, :])
        x8["tg"] = t
        nc.sync.dma_start(x8["bg"][:, 20:RX, :], d_x8bg[:, 20:RX, :])
        nc.gpsimd.dma_start(x8["tg"][:, 20:RX, :], d_x8tg[:, 20:RX, :])
        nc.scalar.dma_start(w1all[:, 0:512], d_w1all[:, 0:512])
        nc.scalar.dma_start(w1all[:, 960:1024], d_w1all[:, 960:1024])
        # packed col offsets: bg0,bg1,bg2 then tg0,tg1,tg2
        W1OFF = {("bg", 0): 0, ("bg", 1): 192, ("bg", 2): 512,
                 ("tg", 0): 960, ("tg", 1): 984, ("tg", 2): 1024}

        def w1ap(s, n, j):
            M = NBG if s == "bg" else NTG
            off = W1OFF[(s, n)] + j * M
            return w1all[0:16 * PS[n], off:off + M]

        ones_l = P.tile([73, 72], BF16, tag="ones")
        nc.gpsimd.dma_start(ones_l[:], d_ones[:])

        # per-branch fused fold outputs: [0:16]=bg, [16:32]=zeros, [32:48]=tg
        fdiv48 = [P.tile([48, BH, W], F16, tag=f"fdiv{n}", name=f"fdiv{n}")
                  for n in range(3)]
        pacc = P.tile([48, 24], F32, tag="pacc")
        late = {}   # deferred persistent tiles: w2bg, w2tg, rdiv48, selw

        state = {}

        def load_late():
            t = P.tile([128, 9, 128], F16, tag="w2bg")
            nc.scalar.dma_start(t[:], d_w2bg[:])
            late["w2bg"] = t
            t = P.tile([64, 3, 128], F16, tag="w2tg")
            nc.scalar.dma_start(t[:], d_w2tg[:])
            late["w2tg"] = t
            t = P.tile([48, 3, BH, W], F16, tag="rdiv")
            nc.scalar.dma_start(t[:], d_rdiv[:])
            late["rdiv"] = t
            t = P.tile([128, 3, 48], F16, tag="selw")
            nc.scalar.dma_start(t[:], d_selw[:])
            late["selw"] = t

        # Replica DMA queue choice per branch avoids queue-cumulative
        # horizon stalls: a conv2 stage must never (transitively) wait on a
        # LATER branch's replica DMAs. Branch 1's replicas are emitted whole
        # after conv2(0); branches 0/2 emit row chunks inline with the muls
        # that produce them — branch 2 on the vector queue, which no earlier
        # conv2 stage depends on.
        REPQ = {0: "sync", 1: None, 2: "gpsimd"}

        def conv1(k):
            n = SEQ[k]
            p, pad, ra = PS[n], PADS[n], RAS[n]
            rxo = 6 - 2 * pad
            wins = _windows(ra)
            nw = len(wins)
            E = pE.tile([73, RA, W], BF16, tag="E")
            nc.gpsimd.dma_start(E[72:73, 0:ra, :], d_hug[n:n + 1, 0:ra, :])
            Sbg = pS.tile([128, RA, 136], F16, tag="Sbg")
            Stg = pS.tile([72, RA, 144], F16, tag="Stg")
            nc.gpsimd.memset(Sbg[:, :, 0:4], 0.0)
            nc.gpsimd.memset(Sbg[:, :, 131:136], 0.0)
            nc.gpsimd.memset(Stg[:, :, 0:15], 0.0)
            nc.gpsimd.memset(Stg[:, :, 136:144], 0.0)
            repq = getattr(nc, REPQ[k]) if REPQ[k] else None

            def denom_pair(w0):
                # denominator + softmax muls for windows w0, w0+1 (<=8 rows)
                r0 = wins[w0][0]
                rr8 = wins[w0][1] + (wins[w0 + 1][1] if w0 + 1 < nw else 0)
                rcp = pRcp.tile([72, 8, W], F32, tag="rcp")
                segs = [(0, wins[w0][1])]
                if w0 + 1 < nw:
                    segs.append((wins[w0][1], wins[w0 + 1][1]))
                for h, rr in segs:
                    den = ps_c2.tile([72, 4, W], F32, tag="c2", name="den")
                    nc.tensor.matmul(den[0:72, 0:rr, :], ones_l[:, 0:72],
                                     E[:, r0 + h:r0 + h + rr, :],
                                     start=True, stop=True)
                    nc.vector.reciprocal_approx_fast(rcp[0:72, h:h + rr, :],
                                                     den[0:72, 0:rr, :])
                nc.vector.tensor_mul(Sbg[0:64, r0:r0 + rr8, 3:131],
                                     E[0:64, r0:r0 + rr8, :],
                                     rcp[0:64, 0:rr8, :])
                nc.gpsimd.tensor_mul(Stg[64:72, r0:r0 + rr8, 8:136],
                                     E[64:72, r0:r0 + rr8, :],
                                     rcp[64:72, 0:rr8, :])
                # bg replica chunk for exactly these rows rides immediately
                if repq is not None:
                    repq.dma_start(Sbg[64:128, r0:r0 + rr8, 4:132],
                                   Sbg[0:64, r0:r0 + rr8, 3:131])

            for w, (r0, rr) in enumerate(wins):
                # blocked emission (interleaving tile-positions stalls the PE
                # pipeline — probes A/D); tg goes to partitions 64:72 via
                # tile_position so its exp writes E[64:72] directly (no
                # cross-partition DMA). C1SHARED: one PSUM bank for both
                # groups sequentially -> one merged exp.
                if C1SHARED:
                    stB = ps_c1.tile([72, 4, W], F32, tag="c1")
                    stT = stB
                else:
                    stB = ps_c1.tile([64, 4, W], F32, tag="c1")
                    stT = ps_c1.tile([72, 4, W], F32, tag="c1", name="stT")
                for j in range(p):
                    nc.tensor.matmul(
                        stB[0:64, 0:rr, :],
                        w1ap("bg", n, j),
                        x8["bg"][0:16 * p, r0 + rxo:r0 + rxo + rr,
                                 j + 3 - pad:j + 3 - pad + W],
                        start=(j == 0), stop=(j == p - 1),
                        skip_group_check=True)
                for j in range(p):
                    nc.tensor.matmul(
                        stT[64:72, 0:rr, :],
                        w1ap("tg", n, j),
                        x8["tg"][0:16 * p, r0 + rxo:r0 + rxo + rr,
                                 j + 3 - pad:j + 3 - pad + W],
                        start=(j == 0), stop=(j == p - 1),
                        tile_position=(0, 64),
                        skip_group_check=True)
                if C1SHARED:
                    nc.scalar.activation(E[0:72, r0:r0 + rr, :],
                                         stB[0:72, 0:rr, :],
                                         mybir.ActivationFunctionType.Exp)
                else:
                    nc.scalar.activation(E[0:64, r0:r0 + rr, :],
                                         stB[0:64, 0:rr, :],
                                         mybir.ActivationFunctionType.Exp)
                    nc.scalar.activation(E[64:72, r0:r0 + rr, :],
                                         stT[64:72, 0:rr, :],
                                         mybir.ActivationFunctionType.Exp)
                if (w % 2 == 1 or w == nw - 1) and w >= 3:
                    denom_pair(w - 2 - (w % 2))
                # deferred loads ride behind the first windows
                if k == 0 and w == 2:
                    load_late()
            # remaining denominator pair
            denom_pair((nw - 1) - ((nw - 1) % 2))
            state[k] = (Sbg, Stg)

        def replicas_tg(k):
            # tg replica DMAs (whole branch, p col-shifted copies)
            n = SEQ[k]
            p, ra = PS[n], RAS[n]
            _, Stg = state[k]
            repq = getattr(nc, REPQ[k] or "sync")
            for g in range(p):
                repq.dma_start(Stg[8 * g:8 * g + 8, 0:ra, 8 + g:136 + g],
                               Stg[64:72, 0:ra, 8:136])

        def replicas_bg(k):
            # whole-branch bg replica (branch 1: emitted after conv2(0) so
            # conv2(0)'s sync horizon never includes it)
            n = SEQ[k]
            ra = RAS[n]
            Sbg, _ = state[k]
            nc.sync.dma_start(Sbg[64:128, 0:ra, 4:132],
                              Sbg[0:64, 0:ra, 3:131])

        def fold_dma(k):
            # fold_y stage 1: align each group's rows with per-group DMAs
            # (only DMAs can shift rows per partition group); emitted right
            # after conv2(k) so the gpsimd queue runs these before the next
            # branch's fold needs the data.
            n = SEQ[k]
            p = PS[n]
            Rs = state[k]
            Q = {}
            for si in range(2):
                Q[si] = pQ.tile([128, BH, W], F16, tag="Q", name=f"Q{si}")
                for g in range(p):
                    nc.sync.dma_start(Q[si][16 * g:16 * g + 16, :, :],
                                      Rs[si][16 * g:16 * g + 16, g:g + BH, :])
            state[("Q", k)] = Q

        def fold_mm_gen(k):
            # fold_y stage 2: contract the groups with a 0/1 selection matrix.
            # bg -> PSUM[0:16] (tile 0,0), tg -> PSUM[32:48] (tile 0,32), both
            # in ONE bank; drained by one fused STT (x rdiv, accum -> pool).
            # PSUM rows 16:32 are stale-but-finite; rdiv is 0 there.
            n = SEQ[k]
            p = PS[n]
            Q = state[("Q", k)]
            sel = late["selw"]
            rdiv = late["rdiv"]
            for ci, r0 in enumerate(range(0, BH, 4)):
                rpf = ps_c2.tile([48, 4, W], F32, tag="c2", name="rpf")
                nc.tensor.matmul(rpf[0:48, :, :], sel[0:16 * p, n, 0:48],
                                 Q[0][0:16 * p, r0:r0 + 4, :],
                                 start=True, stop=True, skip_group_check=True)
                nc.tensor.matmul(rpf[32:48, :, :], sel[0:16 * p, n, 0:16],
                                 Q[1][0:16 * p, r0:r0 + 4, :],
                                 start=True, stop=True,
                                 tile_position=(0, 32), skip_group_check=True)
                # gpsimd cannot read PSUM -> all fused drains on vector
                nc.vector.scalar_tensor_tensor(
                    fdiv48[n][:, r0:r0 + 4, :],
                    rpf[:, :, :], 0.0,
                    rdiv[:, n, r0:r0 + 4, :],
                    op0=mybir.AluOpType.bypass, op1=mybir.AluOpType.mult,
                    accum_out=pacc[:, 8 * n + ci:8 * n + ci + 1])
                yield

        def conv2(k, foldgen=None):
            n = SEQ[k]
            p, pad, ra = PS[n], PADS[n], RAS[n]
            Sbg, Stg = state[k]
            w2bg, w2tg = late["w2bg"], late["w2tg"]
            Rbg = pR.tile([128, RA, W], F16, tag="R")
            Rtg = pR.tile([128, RA, W], F16, tag="R")
            nchk = (p + 1) // 2
            for wi, (r0, rr) in enumerate(_windows(ra)):
                rp = ps_c2.tile([128, 4, W], F32, tag="c2")
                for ci in range(nchk):
                    jj = 2 * ci
                    nc.tensor.matmul(
                        rp[:, 0:rr, :],
                        w2bg[:, W2BASE[n] + ci, :],
                        Sbg[:, r0:r0 + rr, 3 + pad - jj:3 + pad - jj + W],
                        start=(ci == 0), stop=(ci == nchk - 1))
                nc.scalar.activation(Rbg[:, r0:r0 + rr, :], rp[:, 0:rr, :],
                                     mybir.ActivationFunctionType.Copy)
                rp2 = ps_c2.tile([128, 4, W], F32, tag="c2")
                nc.tensor.matmul(rp2[0:128, 0:rr, :],
                                 w2tg[0:8 * p, n, :],
                                 Stg[0:8 * p, r0:r0 + rr, 8 + pad:8 + pad + W],
                                 start=True, stop=True)
                if (r0 // 4) % 2 == 0:
                    nc.scalar.activation(Rtg[:, r0:r0 + rr, :], rp2[:, 0:rr, :],
                                         mybir.ActivationFunctionType.Copy)
                else:
                    nc.vector.tensor_copy(Rtg[:, r0:r0 + rr, :],
                                          rp2[:, 0:rr, :])
                if foldgen is not None and wi >= 2:
                    next(foldgen, None)
            if foldgen is not None:
                for _ in foldgen:
                    pass
            state[k] = (Rbg, Rtg)

        def drain(gen):
            for _ in gen:
                pass

        # ---------------- pipelined emission ----------------
        conv1(0)
        replicas_tg(0)
        conv1(1)
        conv2(0)
        replicas_bg(1)
        replicas_tg(1)
        fold_dma(0)
        conv1(2)
        replicas_tg(2)
        conv2(1, foldgen=fold_mm_gen(0))
        fold_dma(1)
        conv2(2, foldgen=fold_mm_gen(1))
        fold_dma(2)
        drain(fold_mm_gen(2))

        # deferred fdiv stores (bg piece + tg piece per branch)
        for k in range(3):
            n = SEQ[k]
            nc.sync.dma_start(d_fdiv[32 * n:32 * n + 16, :, :],
                              fdiv48[n][0:16, :, :])
            nc.sync.dma_start(d_fdiv[32 * n + 16:32 * n + 32, :, :],
                              fdiv48[n][32:48, :, :])
        nc.sync.dma_start(d_pool[:], pacc[:])

    nc.compile()
    return nc


def _build_B():
    nc = bacc.Bacc("TRN2", target_bir_lowering=False, debug=False,
                   num_devices=NCORES)
    d_f = nc.dram_tensor("fdiv_in", [96, BH, W], F16, kind="ExternalInput")
    d_wt = nc.dram_tensor("wt96", [96, 1], F32, kind="ExternalInput")
    d_obg = nc.dram_tensor("out_bg", [C, BH, W], F16, kind="ExternalOutput")
    d_otg = nc.dram_tensor("out_tg", [C, BH, W], F16, kind="ExternalOutput")

    with tile.TileContext(nc) as tc, ExitStack() as ctx:
        Q = ctx.enter_context(tc.tile_pool(name="q", bufs=1))
        fdv = Q.tile([96, BH, W], F16, tag="fdv")
        wt = Q.tile([96, 1], F32, tag="wt")
        gb = Q.tile([32, BH, W], F16, tag="gb")
        gc = Q.tile([32, BH, W], F16, tag="gc")
        nc.sync.dma_start(wt[:], d_wt[:])
        # row-halved software pipeline: load / scale / gather / add / store
        eng = (nc.sync, nc.scalar)
        for h in range(2):
            r = slice(16 * h, 16 * h + 16)
            eng[h].dma_start(fdv[:, r, :], d_f[:, r, :])
        for h in range(2):
            r = slice(16 * h, 16 * h + 16)
            nc.vector.tensor_scalar_mul(fdv[:, r, :], fdv[:, r, :], wt[:])
            eng[h].dma_start(gb[:, r, :], fdv[32:64, r, :])
            eng[1 - h].dma_start(gc[:, r, :], fdv[64:96, r, :])
            nc.vector.tensor_add(fdv[0:32, r, :], fdv[0:32, r, :],
                                 gb[:, r, :])
            nc.vector.tensor_add(fdv[0:32, r, :], fdv[0:32, r, :],
                                 gc[:, r, :])
            eng[h].dma_start(d_obg[:, r, :], fdv[0:16, r, :])
            eng[1 - h].dma_start(d_otg[:, r, :], fdv[16:32, r, :])

    nc.compile()
    return nc


# ======================= host-side prep =======================

def _prep_core(inputs, b, k):
    y0 = BH * k
    m = {}
    for s, key in (("bg", "bg"), ("tg", "tg")):
        x = np.asarray(inputs[key])[b]          # [C, H, W]
        x8 = np.zeros((7, C, RX, CX), np.float32)
        for g in range(7):
            lo = y0 - 6 + g
            hi = lo + RX
            slo, shi = max(lo, 0), min(hi, H)
            if slo < shi:
                x8[g, :, slo - lo:shi - lo, 3:131] = x[:, slo:shi, :]
        m[f"x8{s}"] = x8.reshape(112, RX, CX).astype(np.float16)

    hug = np.zeros((3, RA, W), np.float32)
    for n, pad in enumerate(PADS):
        for r in range(RA):
            y = y0 - pad + r
            if not (0 <= y < H):
                hug[n, r, :] = 1e30
    m["hug"] = hug

    # fused fold drain: [0:16]=bg rdiv, [16:32]=0 (kills stale PSUM), [32:48]=tg
    rdiv = np.zeros((48, 3, BH, W), np.float32)
    for n, pad in enumerate(PADS):
        yy = np.arange(H)
        rc = np.minimum(yy, pad) + np.minimum(H - 1 - yy, pad) + 1.0
        cc = np.minimum(yy[:W], pad) + np.minimum(W - 1 - yy[:W], pad) + 1.0
        div = np.outer(rc[y0:y0 + BH], cc) + 1e-8
        r = (1.0 / div).astype(np.float32)
        rdiv[0:16, n] = r[None, :, :]
        rdiv[32:48, n] = r[None, :, :]
    m["rdiv"] = rdiv.astype(np.float16)

    ones = np.zeros((73, 72), np.float32)
    ones[0:64, 0:64] = 1.0
    ones[64:72, 64:72] = 1.0
    ones[72, :] = 1.0
    m["oneslhs"] = ones        # cast to bf16 at upload below

    w1all = np.zeros((112, 1080), np.float32)
    w1off = {("bg", 0): 0, ("bg", 1): 192, ("bg", 2): 512,
             ("tg", 0): 960, ("tg", 1): 984, ("tg", 2): 1024}
    for s, M, nmem in (("bg", NBG, "bg_mem"), ("tg", NTG, "tg_mem")):
        for n, p in enumerate(PS):
            mem = np.asarray(inputs[f"{nmem}{n}"])          # [M, C*p*p]
            temp = float(np.asarray(inputs[f"{s}_temp{n}"])[0])
            D = C * p * p
            arr = mem.reshape(M, C, p, p)
            w1 = arr.transpose(2, 1, 3, 0).reshape(p * C, p * M)
            off = w1off[(s, n)]
            w1all[0:16 * p, off:off + p * M] = w1 * (temp / np.sqrt(D))
    m["w1all"] = w1all.astype(np.float16)

    # fold consumes group q at row shift +q where q = 2*pad - i
    w2bg = np.zeros((2, NBG, 9, 8, 16), np.float32)
    for n, p in enumerate(PS):
        pad = PADS[n]
        arr = np.asarray(inputs[f"bg_mem{n}"]).reshape(NBG, C, p, p)
        for ci in range((p + 1) // 2):
            for g in range(2):
                j = 2 * ci + g
                if j < p:
                    for i in range(p):
                        w2bg[g, :, W2BASE[n] + ci, 2 * pad - i, :] = \
                            arr[:, :, i, j]
    m["w2bg"] = w2bg.reshape(128, 9, 128).astype(np.float16)

    w2tg = np.zeros((8, NTG, 3, 8, 16), np.float32)
    for n, p in enumerate(PS):
        pad = PADS[n]
        arr = np.asarray(inputs[f"tg_mem{n}"]).reshape(NTG, C, p, p)
        for g in range(p):
            for i in range(p):
                w2tg[g, :, n, 2 * pad - i, :] = arr[:, :, i, g]
    m["w2tg"] = w2tg.reshape(64, 3, 128).astype(np.float16)

    selw = np.zeros((128, 3, 48), np.float32)
    for n, p in enumerate(PS):
        for g in range(p):
            for c in range(16):
                selw[16 * g + c, n, c] = 1.0
    m["selw"] = selw.astype(np.float16)

    # bf16 upload for the ones matrix
    import ml_dtypes
    m["oneslhs"] = m["oneslhs"].astype(ml_dtypes.bfloat16)
    return m


def _host_mlp(inputs, poolsum):
    """Per batch: pooled -> relu MLP -> softmax over scales -> wt96.

    poolsum: [48, 24] summed over band cores; rows 0:16 = bg per-channel
    partial sums per (branch, chunk) col = 8n+chunk; rows 32:48 = tg.
    """
    wt96 = np.zeros((96, 1), np.float32)
    for si, s in enumerate(("bg", "tg")):
        rows = slice(0, 16) if si == 0 else slice(32, 48)
        pooled = poolsum[rows, :].sum(axis=1) / (H * W)
        w1 = np.asarray(inputs[f"{s}_fc1_w"], np.float64)
        b1 = np.asarray(inputs[f"{s}_fc1_b"], np.float64)
        w2 = np.asarray(inputs[f"{s}_fc2_w"], np.float64)
        b2 = np.asarray(inputs[f"{s}_fc2_b"], np.float64)
        hdn = np.maximum(w1 @ pooled + b1, 0.0)
        logits = (w2 @ hdn + b2).reshape(3, 16)
        e = np.exp(logits - logits.max(axis=0, keepdims=True))
        wt = e / e.sum(axis=0, keepdims=True)
        for n in range(3):
            wt96[32 * n + 16 * si:32 * n + 16 * si + 16, 0] = wt[n]
    return wt96


def kernel(**inputs):
    if "ncA" not in _CACHE:
        _CACHE["ncA"] = _build_A()
        _CACHE["ncB"] = _build_B()

    in_maps = []
    for core in range(NCORES):
        b, k = divmod(core, NBANDS)
        in_maps.append(_prep_core(inputs, b, k))

    resA = run_bass_kernel_spmd(_CACHE["ncA"], in_maps, list(range(NCORES)))

    # host glue: reduce pooled partials within each batch's 4-band group,
    # then the tiny fusion MLP (exact, fp64)
    wt = {}
    for b in range(B):
        poolsum = np.sum([resA.results[b * NBANDS + k]["pool_out"]
                          for k in range(NBANDS)], axis=0).astype(np.float64)
        wt[b] = _host_mlp(inputs, poolsum)

    in_mapsB = []
    for core in range(NCORES):
        b, k = divmod(core, NBANDS)
        in_mapsB.append({
            "fdiv_in": resA.results[core]["fdiv_out"],
            "wt96": wt[b],
        })
    resB = run_bass_kernel_spmd(_CACHE["ncB"], in_mapsB, list(range(NCORES)))

    f_bc = np.zeros((B, C, H, W), np.float32)
    f_tg = np.zeros((B, C, H, W), np.float32)
    for core in range(NCORES):
        b, k = divmod(core, NBANDS)
        y0 = BH * k
        f_bc[b, :, y0:y0 + BH, :] = resB.results[core]["out_bg"].astype(np.float32)
        f_tg[b, :, y0:y0 + BH, :] = resB.results[core]["out_tg"].astype(np.float32)
    return (f_bc, f_tg)
